# revision 7
# baseline (speedup 1.0000x reference)
"""NRI MLP Encoder on 8 Trainium2 NeuronCores (Bass/Tile).

Strategy: data-parallel over batch B (8 samples per core). All MLP weights
replicated. Activations kept feature-major ([H, cols] with features on SBUF
partitions). Edge-level work runs on the full 64x64 node grid per sample
(4096 cols incl. self-loops, which are excluded exactly from the BatchNorm
statistics and the edge->node scatter, and dropped host-side at the end).

node2edge gathers are expressed as broadcast access patterns directly in the
matmul moving operand, so they cost zero extra passes. ELU is computed
exactly as  elu(z)+1 = max(z+b+1, min(exp(z+b), 1))  via one ScalarE Exp
pass plus one VectorE scalar_tensor_tensor(min,max) pass; the "+1" shift is
absorbed into the next layer's bias (b2' = b2 - colsum(W2)). Biases enter
PSUM through K=1 bias-row matmuls. BatchNorm (training stats over the full
batch) uses per-core partial sums + a [128,2] AllReduce per BN layer; the
resulting scale/shift is folded into downstream weights on device.
"""

import sys

sys.path.insert(0, "/opt/trn_rl_repo")

import numpy as np

B, N, T, D = 64, 64, 49, 4
H = 128
TD = T * D            # 196
E_REAL = N * (N - 1)  # 4032
GRID = N * N          # 4096 (incl self)
N_CORES = 8
B_LOC = B // N_CORES  # 8
NODE_COLS = B_LOC * N       # 512 node cols per core
GCOLS = B_LOC * GRID        # 32768 grid cols per core
TILE = 512
NT = GCOLS // TILE          # 64 grid tiles per core
BLK = TILE // N             # 8 receiver blocks per tile
EPS = 1e-5

M_NODE = B * N              # 4096 rows for node-level BNs
M_EDGE = B * E_REAL         # 258048 rows for edge-level BNs

_CACHE = {}


def _build():
    import concourse.bass as bass
    import concourse.mybir as mybir
    from concourse import bacc, tile

    F32 = mybir.dt.float32
    F32R = mybir.dt.float32r
    AF = mybir.ActivationFunctionType
    OP = mybir.AluOpType

    nc = bacc.Bacc("TRN2", target_bir_lowering=False, debug=False,
                   num_devices=N_CORES)

    def din(name, shape, dt=F32R):
        return nc.dram_tensor(name, list(shape), dt, kind="ExternalInput")

    # per-core activations input (this core's 8 samples), feature-major,
    # split at K=128 for the 196(+1 ones row)-row contraction
    xa_d = din("xa", [128, NODE_COLS])           # rows 0:128 of [x; ones]
    xb_d = din("xb", [69, NODE_COLS])            # rows 128:197 (68 feat + ones)
    # packed weights: 12 [H,H] mats + fcw [H,4], one DMA
    wbig_d = din("wbig", [H, 12 * H + 4])
    w_emb1b_d = din("w_emb1b", [69, H])          # last row = emb_b1 + 1
    # packed rows: 6 bias rows + ones(TILE) + fcb(4), one DMA
    rowpack_d = din("rowpack", [1, 6 * H + TILE + 4])
    # packed per-partition vectors, one DMA
    v_names = ["emb_g", "emb_bt", "n2e_g", "n2e_bt", "e2n_g", "e2n_bt",
               "out_g", "out_bt", "out_b1p1"]
    vpack_d = din("vpack", [H, len(v_names)], F32)

    out_d = nc.dram_tensor("out", [GCOLS, 4], F32, kind="ExternalOutput")

    RG = [list(range(N_CORES))]

    with tile.TileContext(nc) as tc:
        with (
            tc.tile_pool(name="wpool", bufs=1) as wp,
            tc.tile_pool(name="grid", bufs=NT) as gp,
            tc.tile_pool(name="work", bufs=3) as wk,
            tc.tile_pool(name="small", bufs=1) as sm,
            tc.tile_pool(name="ps_a", bufs=3, space="PSUM") as ps_a,
            tc.tile_pool(name="ps_b", bufs=3, space="PSUM") as ps_b,
            tc.tile_pool(name="ps_s", bufs=1, space="PSUM") as ps_s,
            tc.tile_pool(name="ps_f", bufs=1, space="PSUM") as ps_f,
            tc.tile_pool(name="dram", bufs=1, space="DRAM") as dp,
        ):
            # ---- load weights / constants ------------------------------
            def ld(d, shape, dt=F32R):
                t = wp.tile(list(shape), dt, tag=d.name)
                nc.sync.dma_start(t[:], d[:, :])
                return t

            xa = ld(xa_d, [128, NODE_COLS])
            xb = ld(xb_d, [69, NODE_COLS])
            wbig = ld(wbig_d, [H, 12 * H + 4])
            w_emb1b = ld(w_emb1b_d, [69, H])
            rowpack = ld(rowpack_d, [1, 6 * H + TILE + 4])
            vpack = ld(vpack_d, [H, len(v_names)], F32)
            (w_emb1a, w_emb2, w_n2e_r, w_n2e_s, w_n2e2, w_e2n1, w_e2n2,
             w_out_r, w_out_s, w_out_skip, w_out2, ident) = [
                wbig[:, H * i: H * (i + 1)] for i in range(12)]
            fcw = wbig[:, 12 * H: 12 * H + 4]
            (r_emb2, r_n2e1, r_n2e2, r_e2n1, r_e2n2, r_out2) = [
                rowpack[:, H * i: H * (i + 1)] for i in range(6)]
            ones = rowpack[:, 6 * H: 6 * H + TILE]
            fcb = rowpack[:, 6 * H + TILE: 6 * H + TILE + 4]
            v = {k: vpack[:, i: i + 1] for i, k in enumerate(v_names)}

            neg1 = sm.tile([H, 1], F32, tag="neg1")
            nc.vector.memset(neg1[:], -1.0)
            epst = sm.tile([H, 1], F32, tag="epst")
            nc.vector.memset(epst[:], EPS)

            def elu_tile(zp, ncols, out_dt=F32R, tag="y"):
                """y~ = max(z, min(exp(z-1), 1)); PSUM z already holds b+1."""
                Et = wk.tile([H, ncols], F32, tag="E")
                nc.scalar.activation(Et[:], zp[:], AF.Exp, bias=neg1[:], scale=1.0)
                y = wk.tile([H, ncols], out_dt, tag=tag)
                nc.vector.scalar_tensor_tensor(
                    y[:], Et[:], 1.0, zp[:], op0=OP.min, op1=OP.max)
                return y

            def mlp_node(xt_list, w1_list, row1, w2, row2, ncols, tag):
                """Two-layer MLP at node level. xt_list/w1_list: K-split rhs/lhsT."""
                zp = ps_a.tile([H, ncols], F32, tag="zpa")
                nk = len(xt_list)
                for i, (xt, w1) in enumerate(zip(xt_list, w1_list)):
                    nc.tensor.matmul(zp[:], w1, xt, start=(i == 0),
                                     stop=(row1 is None and i == nk - 1))
                if row1 is not None:
                    nc.tensor.matmul(zp[:], row1[:, :], ones[:, 0:ncols],
                                     start=False, stop=True)
                y1 = elu_tile(zp, ncols, tag=f"{tag}_y1")
                zp2 = ps_b.tile([H, ncols], F32, tag="zpb")
                nc.tensor.matmul(zp2[:], w2, y1[:], start=True, stop=False)
                nc.tensor.matmul(zp2[:], row2[:, :], ones[:, 0:ncols],
                                 start=False, stop=True)
                return elu_tile(zp2, ncols, tag=f"{tag}_y2")

            # sum/sumsq [H,2] from a bn_aggr [mean,var] and count
            def sums_from_mv(mv, count, dst):
                m2 = sm.tile([H, 1], F32, tag=f"{dst.name}_m2")
                nc.vector.tensor_tensor(m2[:], mv[:, 0:1], mv[:, 0:1], op=OP.mult)
                nc.vector.tensor_tensor(m2[:], mv[:, 1:2], m2[:], op=OP.add)
                nc.vector.tensor_scalar_mul(dst[:, 0:1], mv[:, 0:1], float(count))
                nc.vector.tensor_scalar_mul(dst[:, 1:2], m2[:], float(count))

            def allreduce2(src, tag):
                ci = dp.tile([H, 2], F32, tag=f"cc_in_{tag}")
                co = dp.tile([H, 2], F32, tag=f"cc_out_{tag}")
                nc.sync.dma_start(ci[:], src[:])
                nc.gpsimd.collective_compute(
                    "AllReduce", OP.add, replica_groups=RG,
                    ins=[ci[:].opt()], outs=[co[:].opt()])
                dst = sm.tile([H, 2], F32, tag=f"cc_res_{tag}")
                nc.sync.dma_start(dst[:], co[:])
                return dst

            # global [sum, sumsq] (count M) -> s = g*rsqrt(var+eps), t = bt - mu*s
            def bn_fold(glob, M, g_v, bt_v, tag):
                mu = sm.tile([H, 1], F32, tag=f"mu_{tag}")
                nc.vector.tensor_scalar_mul(mu[:], glob[:, 0:1], 1.0 / M)
                ex2 = sm.tile([H, 1], F32, tag=f"ex2_{tag}")
                nc.vector.tensor_scalar_mul(ex2[:], glob[:, 1:2], 1.0 / M)
                musq = sm.tile([H, 1], F32, tag=f"musq_{tag}")
                nc.vector.tensor_tensor(musq[:], mu[:], mu[:], op=OP.mult)
                var = sm.tile([H, 1], F32, tag=f"var_{tag}")
                nc.vector.tensor_tensor(var[:], ex2[:], musq[:], op=OP.subtract)
                sd = sm.tile([H, 1], F32, tag=f"sd_{tag}")
                nc.scalar.activation(sd[:], var[:], AF.Sqrt, bias=epst[:], scale=1.0)
                inv = sm.tile([H, 1], F32, tag=f"inv_{tag}")
                nc.vector.reciprocal(inv[:], sd[:])
                s = sm.tile([H, 1], F32, tag=f"s_{tag}")
                nc.vector.tensor_tensor(s[:], g_v[:], inv[:], op=OP.mult)
                t = sm.tile([H, 1], F32, tag=f"t_{tag}")
                nc.vector.tensor_tensor(t[:], mu[:], s[:], op=OP.mult)
                nc.vector.tensor_tensor(t[:], bt_v[:], t[:], op=OP.subtract)
                return s, t

            # =========== Phase A: emb MLP (node level, this core) =========
            y2emb = mlp_node([xa[:], xb[:]], [w_emb1a, w_emb1b[:]], None,
                             w_emb2, r_emb2, NODE_COLS, "emb")
            st_emb = sm.tile([H, 6], F32, tag="st_emb")
            nc.vector.bn_stats(st_emb[:], y2emb[:])
            mv_emb = sm.tile([H, 2], F32, tag="mv_emb")
            nc.vector.bn_aggr(mv_emb[:], st_emb[:])
            part_emb = sm.tile([H, 2], F32, tag="part_emb")
            sums_from_mv(mv_emb, NODE_COLS, part_emb)
            glob_emb = allreduce2(part_emb, "emb")
            s0, t0 = bn_fold(glob_emb, M_NODE, v["emb_g"], v["emb_bt"], "emb")
            x_node = sm.tile([H, NODE_COLS], F32R, tag="x_node")
            nc.vector.tensor_scalar(x_node[:], y2emb[:], s0[:], t0[:],
                                    op0=OP.mult, op1=OP.add)

            # =========== Phase B: n2e MLP over the grid ===================
            stats_n2e = sm.tile([H, NT * 6], F32, tag="stats_n2e")
            Rsum = sm.tile([H, NODE_COLS], F32, tag="Rsum")
            selfb = sm.tile([H, NODE_COLS], F32, tag="selfb")
            grid_slots = []
            for t in range(NT):
                s, k = divmod(t, BLK)
                base = s * N
                recv = x_node[:, base + BLK * k: base + BLK * k + BLK] \
                    .unsqueeze(2).broadcast_to([H, BLK, N])
                send = x_node[:, base: base + N] \
                    .unsqueeze(1).broadcast_to([H, BLK, N])
                zp = ps_a.tile([H, TILE], F32, tag="zpa")
                nc.tensor.matmul(zp[:], w_n2e_r, recv, start=True, stop=False)
                nc.tensor.matmul(zp[:], w_n2e_s, send, start=False, stop=False)
                nc.tensor.matmul(zp[:], r_n2e1, ones, start=False, stop=True)
                y1 = elu_tile(zp, TILE, tag="gy1")
                zp2 = ps_b.tile([H, TILE], F32, tag="zpb")
                nc.tensor.matmul(zp2[:], w_n2e2, y1[:], start=True, stop=False)
                nc.tensor.matmul(zp2[:], r_n2e2, ones, start=False, stop=True)
                Et = wk.tile([H, TILE], F32, tag="E")
                nc.scalar.activation(Et[:], zp2[:], AF.Exp, bias=neg1[:], scale=1.0)
                y2 = gp.tile([H, TILE], F32R, tag="gslot")
                nc.vector.scalar_tensor_tensor(
                    y2[:], Et[:], 1.0, zp2[:], op0=OP.min, op1=OP.max)
                grid_slots.append(y2)
                nc.vector.bn_stats(stats_n2e[:, 6 * t: 6 * t + 6], y2[:])
                # receiver-block sums and self-edge extraction
                nc.vector.reduce_sum(
                    Rsum[:, BLK * t: BLK * t + BLK],
                    y2[:].rearrange("p (i j) -> p i j", j=N),
                    axis=mybir.AxisListType.X)
                sv = y2[:][:, BLK * k: BLK * k + 65 * (BLK - 1) + 1: 65]
                nc.gpsimd.tensor_copy(selfb[:, BLK * t: BLK * t + BLK], sv)

            mv_n2e = sm.tile([H, 2], F32, tag="mv_n2e")
            nc.vector.bn_aggr(mv_n2e[:], stats_n2e[:])
            st_self = sm.tile([H, 6], F32, tag="st_self")
            nc.vector.bn_stats(st_self[:], selfb[:])
            mv_self = sm.tile([H, 2], F32, tag="mv_self")
            nc.vector.bn_aggr(mv_self[:], st_self[:])
            sums_f = sm.tile([H, 2], F32, tag="sums_f")
            sums_from_mv(mv_n2e, GCOLS, sums_f)
            sums_s = sm.tile([H, 2], F32, tag="sums_s")
            sums_from_mv(mv_self, NODE_COLS, sums_s)
            part_n2e = sm.tile([H, 2], F32, tag="part_n2e")
            nc.vector.tensor_tensor(part_n2e[:], sums_f[:], sums_s[:],
                                    op=OP.subtract)
            # R correction can overlap the AllReduce
            glob_n2e = allreduce2(part_n2e, "n2e")
            Rreal = sm.tile([H, NODE_COLS], F32, tag="Rreal")
            nc.vector.tensor_tensor(Rreal[:], Rsum[:], selfb[:], op=OP.subtract)
            s1, t1 = bn_fold(glob_n2e, M_EDGE, v["n2e_g"], v["n2e_bt"], "n2e")

            # =========== Phase C: e2n MLP (node level) ====================
            s1d = sm.tile([H, 1], F32, tag="s1d")
            nc.vector.tensor_scalar_mul(s1d[:], s1[:], 1.0 / N)
            t1d = sm.tile([H, 1], F32, tag="t1d")
            nc.vector.tensor_scalar_mul(t1d[:], t1[:], (N - 1) / N)
            inc_bn = sm.tile([H, NODE_COLS], F32R, tag="inc_bn")
            nc.vector.tensor_scalar(inc_bn[:], Rreal[:], s1d[:], t1d[:],
                                    op0=OP.mult, op1=OP.add)
            y2e = mlp_node([inc_bn[:]], [w_e2n1], r_e2n1, w_e2n2, r_e2n2,
                           NODE_COLS, "e2n")
            st_e2n = sm.tile([H, 6], F32, tag="st_e2n")
            nc.vector.bn_stats(st_e2n[:], y2e[:])
            mv_e2n = sm.tile([H, 2], F32, tag="mv_e2n")
            nc.vector.bn_aggr(mv_e2n[:], st_e2n[:])
            part_e2n = sm.tile([H, 2], F32, tag="part_e2n")
            sums_from_mv(mv_e2n, NODE_COLS, part_e2n)
            glob_e2n = allreduce2(part_e2n, "e2n")

            # overlap AllReduce#2 with skip-weight scaling + bias-row build
            w_skip_s = sm.tile([H, H], F32R, tag="w_skip_s")
            nc.vector.tensor_scalar_mul(w_skip_s[:], w_out_skip, s1[:])
            t1r = sm.tile([H, 2], F32R, tag="t1r")
            nc.vector.tensor_scalar_mul(t1r[:], t1[:].broadcast_to([H, 2]), 1.0)
            cps = ps_s.tile([H, 2], F32, tag="tiny")
            nc.tensor.matmul(cps[:], w_out_skip, t1r[:], start=True, stop=True)
            cfull = sm.tile([H, 1], F32, tag="cfull")
            nc.scalar.activation(cfull[:], cps[:, 0:1], AF.Identity,
                                 bias=v["out_b1p1"], scale=1.0)
            cfr = sm.tile([H, 2], F32R, tag="cfr")
            nc.vector.tensor_scalar_mul(cfr[:], cfull[:].broadcast_to([H, 2]), 1.0)
            rps = ps_s.tile([2, H], F32, tag="tiny")
            nc.tensor.matmul(rps[:], cfr[:], ident, start=True, stop=True)
            r_out1 = sm.tile([1, H], F32R, tag="r_out1")
            nc.vector.tensor_scalar_mul(r_out1[:], rps[0:1, :], 1.0)

            s2, t2 = bn_fold(glob_e2n, M_NODE, v["e2n_g"], v["e2n_bt"], "e2n")
            xn2 = sm.tile([H, NODE_COLS], F32R, tag="xn2")
            nc.vector.tensor_scalar(xn2[:], y2e[:], s2[:], t2[:],
                                    op0=OP.mult, op1=OP.add)

            # =========== Phase D: out MLP over the grid ===================
            stats_out = sm.tile([H, NT * 6], F32, tag="stats_out")
            selfb2 = sm.tile([H, NODE_COLS], F32, tag="selfb2")
            out_slots = []
            for t in range(NT):
                s, k = divmod(t, BLK)
                base = s * N
                recv = xn2[:, base + BLK * k: base + BLK * k + BLK] \
                    .unsqueeze(2).broadcast_to([H, BLK, N])
                send = xn2[:, base: base + N] \
                    .unsqueeze(1).broadcast_to([H, BLK, N])
                zp = ps_a.tile([H, TILE], F32, tag="zpa")
                nc.tensor.matmul(zp[:], w_out_r, recv, start=True, stop=False)
                nc.tensor.matmul(zp[:], w_out_s, send, start=False, stop=False)
                nc.tensor.matmul(zp[:], w_skip_s[:], grid_slots[t][:],
                                 start=False, stop=False)
                nc.tensor.matmul(zp[:], r_out1[:], ones, start=False, stop=True)
                y1 = elu_tile(zp, TILE, tag="gy1")
                zp2 = ps_b.tile([H, TILE], F32, tag="zpb")
                nc.tensor.matmul(zp2[:], w_out2, y1[:], start=True, stop=False)
                nc.tensor.matmul(zp2[:], r_out2, ones, start=False, stop=True)
                Et = wk.tile([H, TILE], F32, tag="E")
                nc.scalar.activation(Et[:], zp2[:], AF.Exp, bias=neg1[:], scale=1.0)
                y2 = gp.tile([H, TILE], F32R, tag="gslot")
                nc.vector.scalar_tensor_tensor(
                    y2[:], Et[:], 1.0, zp2[:], op0=OP.min, op1=OP.max)
                out_slots.append(y2)
                nc.vector.bn_stats(stats_out[:, 6 * t: 6 * t + 6], y2[:])
                sv = y2[:][:, BLK * k: BLK * k + 65 * (BLK - 1) + 1: 65]
                nc.gpsimd.tensor_copy(selfb2[:, BLK * t: BLK * t + BLK], sv)

            mv_out = sm.tile([H, 2], F32, tag="mv_out")
            nc.vector.bn_aggr(mv_out[:], stats_out[:])
            st_self2 = sm.tile([H, 6], F32, tag="st_self2")
            nc.vector.bn_stats(st_self2[:], selfb2[:])
            mv_self2 = sm.tile([H, 2], F32, tag="mv_self2")
            nc.vector.bn_aggr(mv_self2[:], st_self2[:])
            sums_f2 = sm.tile([H, 2], F32, tag="sums_f2")
            sums_from_mv(mv_out, GCOLS, sums_f2)
            sums_s2 = sm.tile([H, 2], F32, tag="sums_s2")
            sums_from_mv(mv_self2, NODE_COLS, sums_s2)
            part_out = sm.tile([H, 2], F32, tag="part_out")
            nc.vector.tensor_tensor(part_out[:], sums_f2[:], sums_s2[:],
                                    op=OP.subtract)
            glob_out = allreduce2(part_out, "out")
            s3, t3 = bn_fold(glob_out, M_EDGE, v["out_g"], v["out_bt"], "out")

            # fc fold: fcw' = diag(s3) @ fcw ; bias4 = t3.T @ fcw + fc_b
            fcw_s = sm.tile([H, 4], F32R, tag="fcw_s")
            nc.vector.tensor_scalar_mul(fcw_s[:], fcw, s3[:])
            t3r = sm.tile([H, 2], F32R, tag="t3r")
            nc.vector.tensor_scalar_mul(t3r[:], t3[:].broadcast_to([H, 2]), 1.0)
            b4ps = ps_s.tile([2, 4], F32, tag="tiny")
            nc.tensor.matmul(b4ps[:], t3r[:], fcw, start=True, stop=True)
            bias4r = sm.tile([1, 4], F32R, tag="bias4r")
            nc.vector.tensor_tensor(bias4r[:], b4ps[0:1, :], fcb, op=OP.add)
            b4rep = ps_s.tile([H, 4], F32, tag="tiny")
            nc.tensor.matmul(b4rep[:], ones[0:1, 0:H], bias4r[:],
                             start=True, stop=True)
            b4sb = sm.tile([H, 4], F32, tag="b4sb")
            nc.vector.tensor_copy(b4sb[:], b4rep[:])

            # =========== Phase E: fc (transposed: edges on partitions) ====
            CH = TILE // H  # 4 chunks of 128 edges per tile
            b4b = b4sb[:].unsqueeze(1).broadcast_to([H, CH, 4])
            outbuf = sm.tile([H, NT * CH * 4], F32, tag="outbuf")
            for t in range(NT):
                fps = ps_f.tile([H, CH * 4], F32, tag="fps")
                for c in range(CH):
                    nc.tensor.matmul(fps[:, 4 * c: 4 * c + 4],
                                     out_slots[t][:, H * c: H * c + H],
                                     fcw_s[:], start=(c == 0), stop=(c == CH - 1))
                nc.vector.tensor_tensor(
                    outbuf[:, CH * 4 * t: CH * 4 * (t + 1)]
                    .rearrange("p (c f) -> p c f", f=4),
                    fps[:].rearrange("p (c f) -> p c f", f=4),
                    b4b, op=OP.add)
            nc.sync.dma_start(
                out_d[:, :].rearrange("(t c p) f -> p t c f", p=H, c=CH),
                outbuf[:].rearrange("p (t c f) -> p t c f", c=CH, f=4))

    nc.finalize()
    return nc


def _prep_static(inputs):
    """Host-side weight packing (independent of per-core slicing)."""
    f32 = np.float32

    def cs(w):  # colsum fold for the +1 shift
        return w.sum(axis=0)

    emb_w1 = np.asarray(inputs["emb_w1"], f32)
    emb_b1 = np.asarray(inputs["emb_b1"], f32)
    emb_w2 = np.asarray(inputs["emb_w2"], f32)
    emb_b2 = np.asarray(inputs["emb_b2"], f32)
    n2e_w1 = np.asarray(inputs["n2e_w1"], f32)
    n2e_b1 = np.asarray(inputs["n2e_b1"], f32)
    n2e_w2 = np.asarray(inputs["n2e_w2"], f32)
    n2e_b2 = np.asarray(inputs["n2e_b2"], f32)
    e2n_w1 = np.asarray(inputs["e2n_w1"], f32)
    e2n_b1 = np.asarray(inputs["e2n_b1"], f32)
    e2n_w2 = np.asarray(inputs["e2n_w2"], f32)
    e2n_b2 = np.asarray(inputs["e2n_b2"], f32)
    out_w1 = np.asarray(inputs["out_w1"], f32)
    out_b1 = np.asarray(inputs["out_b1"], f32)
    out_w2 = np.asarray(inputs["out_w2"], f32)
    out_b2 = np.asarray(inputs["out_b2"], f32)

    wbig = np.concatenate([
        emb_w1[:128], emb_w2,
        n2e_w1[:H], n2e_w1[H:], n2e_w2,
        e2n_w1, e2n_w2,
        out_w1[:H], out_w1[H:2 * H], out_w1[2 * H:], out_w2,
        np.eye(H, dtype=f32),
        np.asarray(inputs["fc_w"], f32),
    ], axis=1)
    w_emb1b = np.concatenate([emb_w1[128:], (emb_b1 + 1.0)[None, :]], 0)
    rowpack = np.concatenate([
        (emb_b2 - cs(emb_w2) + 1.0)[None, :],
        (n2e_b1 + 1.0)[None, :],
        (n2e_b2 - cs(n2e_w2) + 1.0)[None, :],
        (e2n_b1 + 1.0)[None, :],
        (e2n_b2 - cs(e2n_w2) + 1.0)[None, :],
        (out_b2 - cs(out_w2) + 1.0)[None, :],
        np.ones((1, TILE), f32),
        np.asarray(inputs["fc_b"], f32)[None, :],
    ], axis=1)
    vpack = np.stack([
        np.asarray(inputs["emb_g"], f32), np.asarray(inputs["emb_bt"], f32),
        np.asarray(inputs["n2e_g"], f32), np.asarray(inputs["n2e_bt"], f32),
        np.asarray(inputs["e2n_g"], f32), np.asarray(inputs["e2n_bt"], f32),
        np.asarray(inputs["out_g"], f32), np.asarray(inputs["out_bt"], f32),
        (out_b1 + 1.0),
    ], axis=1)
    d = {"wbig": wbig, "w_emb1b": w_emb1b, "rowpack": rowpack, "vpack": vpack}
    return {k: np.ascontiguousarray(val, f32) for k, val in d.items()}


def _check_rel(rel_rec, rel_send):
    rr = np.asarray(rel_rec)
    rs = np.asarray(rel_send)
    recv = rr.argmax(1)
    send = rs.argmax(1)
    i, j = np.where(~np.eye(N, dtype=bool))
    assert rr.shape == (E_REAL, N) and rs.shape == (E_REAL, N)
    assert np.array_equal(recv, i) and np.array_equal(send, j), \
        "rel matrices are not the canonical fully-connected pattern"
    return recv * N + send  # grid column index per real edge


def kernel(**inputs) -> np.ndarray:
    from concourse.bass_utils import run_bass_kernel_spmd

    if "nc" not in _CACHE:
        _CACHE["nc"] = _build()
    nc = _CACHE["nc"]

    cols = _check_rel(inputs["rel_rec"], inputs["rel_send"])
    static = _prep_static(inputs)

    x = np.asarray(inputs["inputs"], np.float32).reshape(B * N, TD)
    x_fm = np.ascontiguousarray(x.T)  # [196, 4096]
    ones_row = np.ones((1, B * N), np.float32)
    x_pack = np.concatenate([x_fm, ones_row], 0)  # [197, 4096]

    in_maps = []
    for c in range(N_CORES):
        sl = slice(c * NODE_COLS, (c + 1) * NODE_COLS)
        m = dict(static)
        m["xa"] = np.ascontiguousarray(x_pack[:128, sl])
        m["xb"] = np.ascontiguousarray(x_pack[128:, sl])
        in_maps.append(m)

    res = run_bass_kernel_spmd(nc, in_maps, core_ids=list(range(N_CORES)),
                               trace=False)
    grid_out = np.concatenate([res.results[c]["out"].reshape(B_LOC, GRID, 4)
                               for c in range(N_CORES)], 0)  # [64, 4096, 4]
    return np.ascontiguousarray(grid_out[:, cols, :])
